# revision 7
# baseline (speedup 1.0000x reference)
"""4-level inverse DWT (db4, symmetric-mode coefficient layout) on TRN2.

Contract: kernel(**inputs) takes FULL inputs (B=64, C=16 batch/channel dims),
returns the FULL (64, 16, 16384) float32 reconstruction.

Sharding: B*C = 1024 signals -> 8 cores x 128 SBUF partitions. Each core runs
the whole 4-level synthesis bank on its 128 signals; no communication.

Math (polyphase form of pywt idwt, valid for this geometry -- no boundary
handling needed): with filter h (rec_lo for the approx branch, rec_hi for the
detail branch), each level computes
    y[2i+p] = sum_{j=0..3} h[(6+p) - 2j] * x[i+j],  i in [0, n-4]
summed over both branches, where n is the (trimmed) coefficient length.
Output length 2n-6; level sizes 1030->2054->4102(->4101)->8196(->8195)->16384.

Engine split (per parity, 8 taps total): the first _PE_TAPS taps run as
scaled-diagonal fp16 matmuls accumulating into PSUM (TensorE does the shifted
multiply-accumulate along the free dim); the remaining taps run on VectorE as
fp32 scalar_tensor_tensor ops, the first consuming PSUM and the last writing
the parity-interleaved output directly. Intermediate level outputs are kept in
fp16 (they only feed the next level's PE taps); the detail inputs stay fp32
for the DVE taps, with fp16 copies made on ScalarE for the PE taps. The final
level accumulates/writes fp32.
"""

import numpy as np

_P = 128
_N_CORES = 8
_IN_LENS = {"approx": 1030, "d0": 1030, "d1": 2054, "d2": 4101, "d3": 8195}
_OUT_LEN = 16384
_CHUNK = 512  # PSUM bank = 512 fp32

_PE_TAPS = 6  # taps per parity on TensorE (0 => pure-DVE fp32 kernel)

# Set by a driving harness (test.py) to collect profile info; harmless default.
_TRACE = False
_LAST_RESULTS = None

_CACHE = {}


def _ensure_paths():
    import sys

    for p in ("/opt/trn_rl_repo", "/root/.axon_site"):
        if p not in sys.path:
            sys.path.insert(0, p)


def _tap_table(lo, hi, p):
    """8 (branch, offset, coef) taps for output parity p, PE-friendly order:
    a-branch taps first, then d-branch."""
    taps = [("a", j, lo[6 + p - 2 * j]) for j in range(4)]
    taps += [("d", j, hi[6 + p - 2 * j]) for j in range(4)]
    return taps


def _build(lo, hi, pe_taps):
    """Compile the per-core Bass kernel. lo/hi are 8-tap float lists."""
    import concourse.tile as tile
    from concourse import bacc, mybir

    f32 = mybir.dt.float32
    f16 = mybir.dt.float16
    mult = mybir.AluOpType.mult
    add = mybir.AluOpType.add

    nc = bacc.Bacc("TRN2", target_bir_lowering=False, debug=False)

    ins = {
        name: nc.dram_tensor(name, [_P, L], f32, kind="ExternalInput").ap()
        for name, L in _IN_LENS.items()
    }
    n_diags = 2 * pe_taps
    if pe_taps:
        diag_ap = nc.dram_tensor(
            "diag", [_P, n_diags * _P], f16, kind="ExternalInput"
        ).ap()
    out_ap = nc.dram_tensor("out", [_P, _OUT_LEN], f32, kind="ExternalOutput").ap()

    # level -> (d name, common length n after trim)
    levels = [("d0", 1030), ("d1", 2054), ("d2", 4101), ("d3", 8195)]

    with tile.TileContext(nc) as tc:
        with (
            tc.tile_pool(name="bufs", bufs=1) as pool,
            tc.tile_pool(name="tmps", bufs=6) as tmp_pool,
            tc.tile_pool(name="psum", bufs=8, space="PSUM") as ps_pool,
        ):
            if pe_taps:
                diag = pool.tile([_P, n_diags * _P], f16, tag="diag")
                nc.sync.dma_start(diag[:], diag_ap)

            # fp32 detail inputs; lifetime-paired slots (d0,d2) and (d1,d3).
            dtag = {"d0": "dA", "d1": "dB", "d2": "dA", "d3": "dB"}
            d32 = {}
            for name in ("d0", "d1", "d2", "d3"):
                t = pool.tile([_P, _IN_LENS[name]], f32, tag=dtag[name])
                nc.sync.dma_start(t[:], ins[name])
                d32[name] = t

            a4_32 = pool.tile([_P, 1030], f32, tag="a4f32")
            nc.sync.dma_start(a4_32[:], ins["approx"])

            if pe_taps:
                # fp16 copies for the PE taps (ScalarE casts)
                a_h = pool.tile([_P, 1030], f16, tag="a4h")
                nc.scalar.copy(a_h[:], a4_32[:])
                d16 = {}
                for name in ("d0", "d1", "d2", "d3"):
                    t = pool.tile([_P, _IN_LENS[name]], f16, tag=dtag[name] + "h")
                    nc.scalar.copy(t[:], d32[name][:])
                    d16[name] = t
            else:
                a_f = a4_32

            for lvl, (dname, n) in enumerate(levels):
                m = n - 3
                last = lvl == len(levels) - 1
                out_dt = f32 if (last or not pe_taps) else f16
                ot = pool.tile([_P, 2 * m], out_dt, tag=f"lv{lvl}")

                chunks = [
                    (c0, min(_CHUNK, m - c0)) for c0 in range(0, m, _CHUNK)
                ]
                # Process chunk-pairs in groups of G so each diag weight is
                # loaded once per G matmuls (tap-outer emission order).
                G = 3
                for g0 in range(0, len(chunks), G):
                    grp = chunks[g0 : g0 + G]
                    pss = {}
                    if pe_taps:
                        for ci in range(len(grp)):
                            for p in (0, 1):
                                t_ps = ps_pool.tile([_P, _CHUNK], f32, tag="ps")
                                pss[(ci, p)] = t_ps
                        for k in range(pe_taps):
                            for p in (0, 1):
                                br, j, _c = _tap_table(lo, hi, p)[k]
                                src = a_h if br == "a" else d16[dname]
                                di = p * pe_taps + k
                                w = diag[:, di * _P : (di + 1) * _P]
                                for ci, (c0, N) in enumerate(grp):
                                    nc.tensor.matmul(
                                        pss[(ci, p)][:, :N],
                                        w,
                                        src[:, c0 + j : c0 + j + N],
                                        start=(k == 0),
                                        stop=(k == pe_taps - 1),
                                    )
                    for ci, (c0, N) in enumerate(grp):
                        for p in (0, 1):
                            self_taps = _tap_table(lo, hi, p)
                            if pe_taps:
                                acc = pss[(ci, p)][:, :N]
                                dve_taps = self_taps[pe_taps:]
                            else:
                                acc = None
                                dve_taps = self_taps

                            out_slice = ot[:, 2 * c0 + p : 2 * (c0 + N) + p - 1 : 2]
                            for k, (br, j, c) in enumerate(dve_taps):
                                src32 = a_f if br == "a" else d32[dname]
                                in0 = src32[:, c0 + j : c0 + j + N]
                                final = k == len(dve_taps) - 1
                                if final:
                                    dst = out_slice
                                else:
                                    tmp_t = tmp_pool.tile(
                                        [_P, _CHUNK], f32, tag="tmp"
                                    )
                                    dst = tmp_t[:, :N]
                                if acc is None:
                                    nc.vector.tensor_scalar_mul(dst, in0, c)
                                else:
                                    nc.vector.scalar_tensor_tensor(
                                        dst, in0, c, acc, mult, add
                                    )
                                acc = dst

                        if last:
                            nc.sync.dma_start(
                                out_ap[:, 2 * c0 : 2 * (c0 + N)],
                                ot[:, 2 * c0 : 2 * (c0 + N)],
                            )

                if pe_taps:
                    a_h = ot if not last else None
                else:
                    a_f = ot

    nc.compile()
    return nc


def kernel(approx, d0, d1, d2, d3, rec_lo, rec_hi):
    _ensure_paths()
    global _LAST_RESULTS
    from concourse.bass_utils import run_bass_kernel_spmd

    lo = [float(v) for v in np.asarray(rec_lo, np.float32)]
    hi = [float(v) for v in np.asarray(rec_hi, np.float32)]
    key = (tuple(lo), tuple(hi), _PE_TAPS)
    if key not in _CACHE:
        _CACHE[key] = _build(lo, hi, _PE_TAPS)
    nc = _CACHE[key]

    arrs = {"approx": approx, "d0": d0, "d1": d1, "d2": d2, "d3": d3}
    flat = {}
    B, C = None, None
    for name, x in arrs.items():
        x = np.asarray(x, np.float32)
        B, C = x.shape[0], x.shape[1]
        flat[name] = np.ascontiguousarray(x.reshape(B * C, x.shape[-1]))

    in_maps = [
        {name: v[i * _P : (i + 1) * _P] for name, v in flat.items()}
        for i in range(_N_CORES)
    ]
    if _PE_TAPS:
        dg = np.zeros((_P, 2 * _PE_TAPS * _P), np.float16)
        eye = np.eye(_P, dtype=np.float64)
        for p in (0, 1):
            for k, (_br, _j, c) in enumerate(_tap_table(lo, hi, p)[:_PE_TAPS]):
                di = p * _PE_TAPS + k
                dg[:, di * _P : (di + 1) * _P] = (eye * c).astype(np.float16)
        for im in in_maps:
            im["diag"] = dg

    res = run_bass_kernel_spmd(nc, in_maps, list(range(_N_CORES)), trace=_TRACE)
    _LAST_RESULTS = res
    out = np.concatenate([res.results[i]["out"] for i in range(_N_CORES)], axis=0)
    return np.ascontiguousarray(out.reshape(B, C, _OUT_LEN).astype(np.float32))


# revision 8
# speedup vs baseline: 1.0551x; 1.0551x over previous
"""4-level inverse DWT (db4, symmetric-mode coefficient layout) on TRN2.

Contract: kernel(**inputs) takes FULL inputs (B=64, C=16 batch/channel dims),
returns the FULL (64, 16, 16384) float32 reconstruction.

Sharding: B*C = 1024 signals -> 8 cores x 128 SBUF partitions. Each core runs
the whole 4-level synthesis bank on its 128 signals; no communication.

Math (polyphase form of pywt idwt, valid for this geometry -- no boundary
handling needed): with filter h (rec_lo for the approx branch, rec_hi for the
detail branch), each level computes
    y[2i+p] = sum_{j=0..3} h[(6+p) - 2j] * x[i+j],  i in [0, n-4]
summed over both branches, where n is the (trimmed) coefficient length.
Output length 2n-6; level sizes 1030->2054->4102(->4101)->8196(->8195)->16384.

Engine split (per parity, 8 taps total): the first _PE_TAPS taps run as
scaled-diagonal fp16 matmuls accumulating into PSUM (TensorE does the shifted
multiply-accumulate along the free dim); the remaining taps run on VectorE as
fp32 scalar_tensor_tensor ops, the first consuming PSUM and the last writing
the parity-interleaved output directly. Intermediate level outputs are kept in
fp16 (they only feed the next level's PE taps); the detail inputs stay fp32
for the DVE taps, with fp16 copies made on ScalarE for the PE taps. The final
level accumulates/writes fp32.
"""

import numpy as np

_P = 128
_N_CORES = 8
_IN_LENS = {"approx": 1030, "d0": 1030, "d1": 2054, "d2": 4101, "d3": 8195}
_OUT_LEN = 16384
_CHUNK = 512  # PSUM bank = 512 fp32

_PE_TAPS = 6  # taps per parity on TensorE (0 => pure-DVE fp32 kernel)

# Set by a driving harness (test.py) to collect profile info; harmless default.
_TRACE = False
_LAST_RESULTS = None

_CACHE = {}


def _ensure_paths():
    import sys

    for p in ("/opt/trn_rl_repo", "/root/.axon_site"):
        if p not in sys.path:
            sys.path.insert(0, p)


def _tap_table(lo, hi, p):
    """8 (branch, offset, coef) taps for output parity p, PE-friendly order:
    a-branch taps first, then d-branch."""
    taps = [("a", j, lo[6 + p - 2 * j]) for j in range(4)]
    taps += [("d", j, hi[6 + p - 2 * j]) for j in range(4)]
    return taps


def _build(lo, hi, pe_taps):
    """Compile the per-core Bass kernel. lo/hi are 8-tap float lists."""
    import concourse.tile as tile
    from concourse import bacc, mybir

    f32 = mybir.dt.float32
    f16 = mybir.dt.float16
    mult = mybir.AluOpType.mult
    add = mybir.AluOpType.add

    nc = bacc.Bacc("TRN2", target_bir_lowering=False, debug=False)

    ins = {
        name: nc.dram_tensor(name, [_P, L], f32, kind="ExternalInput").ap()
        for name, L in _IN_LENS.items()
    }
    n_diags = 2 * pe_taps
    if pe_taps:
        diag_ap = nc.dram_tensor(
            "diag", [_P, n_diags * _P], f16, kind="ExternalInput"
        ).ap()
    out_ap = nc.dram_tensor("out", [_P, _OUT_LEN], f32, kind="ExternalOutput").ap()

    # level -> (d name, common length n after trim)
    levels = [("d0", 1030), ("d1", 2054), ("d2", 4101), ("d3", 8195)]

    with tile.TileContext(nc) as tc:
        with (
            tc.tile_pool(name="bufs", bufs=1) as pool,
            tc.tile_pool(name="tmps", bufs=6) as tmp_pool,
            tc.tile_pool(name="psum", bufs=8, space="PSUM") as ps_pool,
        ):
            # fp16 copies for the PE taps arrive via casting DMAs (gpsimd
            # SWDGE) straight from HBM, in parallel with the fp32 loads on
            # the sync HWDGE queue. Load order puts level-1 inputs first so
            # compute can start as early as possible.
            dtag = {"d0": "dA", "d1": "dB", "d2": "dA", "d3": "dB"}
            d32, d16 = {}, {}

            if pe_taps:
                a_h = pool.tile([_P, 1030], f16, tag="a4h")
                nc.gpsimd.dma_start(a_h[:], ins["approx"])
                t16 = pool.tile([_P, 1030], f16, tag="dAh")
                nc.gpsimd.dma_start(t16[:], ins["d0"])
                d16["d0"] = t16
                diag = pool.tile([_P, n_diags * _P], f16, tag="diag")
                nc.sync.dma_start(diag[:], diag_ap)

            for name in ("d0", "d1", "d2", "d3"):
                t = pool.tile([_P, _IN_LENS[name]], f32, tag=dtag[name])
                nc.sync.dma_start(t[:], ins[name])
                d32[name] = t

            if pe_taps:
                for name in ("d1", "d2", "d3"):
                    t16 = pool.tile([_P, _IN_LENS[name]], f16, tag=dtag[name] + "h")
                    nc.gpsimd.dma_start(t16[:], ins[name])
                    d16[name] = t16
            else:
                a4_32 = pool.tile([_P, 1030], f32, tag="a4f32")
                nc.sync.dma_start(a4_32[:], ins["approx"])
                a_f = a4_32

            for lvl, (dname, n) in enumerate(levels):
                m = n - 3
                last = lvl == len(levels) - 1
                out_dt = f32 if (last or not pe_taps) else f16
                ot = pool.tile([_P, 2 * m], out_dt, tag=f"lv{lvl}")

                chunks = [
                    (c0, min(_CHUNK, m - c0)) for c0 in range(0, m, _CHUNK)
                ]
                # Process chunk-pairs in groups of G so each diag weight is
                # loaded once per G matmuls (tap-outer emission order).
                G = 3
                for g0 in range(0, len(chunks), G):
                    grp = chunks[g0 : g0 + G]
                    pss = {}
                    if pe_taps:
                        for ci in range(len(grp)):
                            for p in (0, 1):
                                t_ps = ps_pool.tile([_P, _CHUNK], f32, tag="ps")
                                pss[(ci, p)] = t_ps
                        for k in range(pe_taps):
                            for p in (0, 1):
                                br, j, _c = _tap_table(lo, hi, p)[k]
                                src = a_h if br == "a" else d16[dname]
                                di = p * pe_taps + k
                                w = diag[:, di * _P : (di + 1) * _P]
                                for ci, (c0, N) in enumerate(grp):
                                    nc.tensor.matmul(
                                        pss[(ci, p)][:, :N],
                                        w,
                                        src[:, c0 + j : c0 + j + N],
                                        start=(k == 0),
                                        stop=(k == pe_taps - 1),
                                    )
                    for ci, (c0, N) in enumerate(grp):
                        for p in (0, 1):
                            self_taps = _tap_table(lo, hi, p)
                            if pe_taps:
                                acc = pss[(ci, p)][:, :N]
                                dve_taps = self_taps[pe_taps:]
                            else:
                                acc = None
                                dve_taps = self_taps

                            out_slice = ot[:, 2 * c0 + p : 2 * (c0 + N) + p - 1 : 2]
                            for k, (br, j, c) in enumerate(dve_taps):
                                src32 = a_f if br == "a" else d32[dname]
                                in0 = src32[:, c0 + j : c0 + j + N]
                                final = k == len(dve_taps) - 1
                                if final:
                                    dst = out_slice
                                else:
                                    tmp_t = tmp_pool.tile(
                                        [_P, _CHUNK], f32, tag="tmp"
                                    )
                                    dst = tmp_t[:, :N]
                                if acc is None:
                                    nc.vector.tensor_scalar_mul(dst, in0, c)
                                else:
                                    nc.vector.scalar_tensor_tensor(
                                        dst, in0, c, acc, mult, add
                                    )
                                acc = dst

                        if last:
                            nc.sync.dma_start(
                                out_ap[:, 2 * c0 : 2 * (c0 + N)],
                                ot[:, 2 * c0 : 2 * (c0 + N)],
                            )

                if pe_taps:
                    a_h = ot if not last else None
                else:
                    a_f = ot

    nc.compile()
    return nc


def kernel(approx, d0, d1, d2, d3, rec_lo, rec_hi):
    _ensure_paths()
    global _LAST_RESULTS
    from concourse.bass_utils import run_bass_kernel_spmd

    lo = [float(v) for v in np.asarray(rec_lo, np.float32)]
    hi = [float(v) for v in np.asarray(rec_hi, np.float32)]
    key = (tuple(lo), tuple(hi), _PE_TAPS)
    if key not in _CACHE:
        _CACHE[key] = _build(lo, hi, _PE_TAPS)
    nc = _CACHE[key]

    arrs = {"approx": approx, "d0": d0, "d1": d1, "d2": d2, "d3": d3}
    flat = {}
    B, C = None, None
    for name, x in arrs.items():
        x = np.asarray(x, np.float32)
        B, C = x.shape[0], x.shape[1]
        flat[name] = np.ascontiguousarray(x.reshape(B * C, x.shape[-1]))

    in_maps = [
        {name: v[i * _P : (i + 1) * _P] for name, v in flat.items()}
        for i in range(_N_CORES)
    ]
    if _PE_TAPS:
        dg = np.zeros((_P, 2 * _PE_TAPS * _P), np.float16)
        eye = np.eye(_P, dtype=np.float64)
        for p in (0, 1):
            for k, (_br, _j, c) in enumerate(_tap_table(lo, hi, p)[:_PE_TAPS]):
                di = p * _PE_TAPS + k
                dg[:, di * _P : (di + 1) * _P] = (eye * c).astype(np.float16)
        for im in in_maps:
            im["diag"] = dg

    res = run_bass_kernel_spmd(nc, in_maps, list(range(_N_CORES)), trace=_TRACE)
    _LAST_RESULTS = res
    out = np.concatenate([res.results[i]["out"] for i in range(_N_CORES)], axis=0)
    return np.ascontiguousarray(out.reshape(B, C, _OUT_LEN).astype(np.float32))
